# revision 1
# baseline (speedup 1.0000x reference)
"""Trainium2 Bass kernel for CustomSTFT (STFT -> mag/phase -> ISTFT roundtrip).

Key insight: magnitude*cos(atan2(i,r)) == r and magnitude*sin(atan2(i,r)) == i
(up to a 1e-14 epsilon inside the sqrt), so the whole pipeline is linear in x:

    y = OLA( frames @ M.T ),   M = w_bwd_r.T @ w_fwd_r - w_bwd_i.T @ w_fwd_i

With hop=200 and n_fft=800, collapsing the frame/OLA structure per 200-sample
chunk gives a 7-band polyphase filter: out_chunk[c] = sum_{d=-3..3} K_d @ in_chunk[c+d]
plus two boundary corrections (first/last output chunk miss one frame).

Sharding: batch 16 across 8 cores (2 samples each). Matmuls in float32r
(full PE rate at N>=256, ~1e-4 relative precision).
"""

import numpy as np

N_CORES = 8
B, T = 16, 240000
SPC = B // N_CORES          # samples per core
N_FFT, HOP, FREQ, PAD = 800, 200, 401, 400
C = HOP                     # chunk size 200
H = C // 2                  # 100: SBUF partition dim (contraction split)
NCH_XP = (T + 2 * PAD) // C     # 1204 chunks in edge-padded signal
NCH_P = NCH_XP + 2              # 1206 incl. one zero chunk each side
NOUT = T // C                   # 1200 output chunks per sample
NTILE = 400                     # output chunks per matmul (psum free dim)
NT = NOUT // NTILE              # 3 interior tiles per sample

_cache = {}


def _host_weights():
    """Build the device weight tensors (float32) from first principles."""
    if "ktt" in _cache:
        return _cache["ktt"], _cache["et"]
    n = np.arange(N_FFT)
    k = np.arange(FREQ)
    win = 0.5 * (1.0 - np.cos(2.0 * np.pi * np.arange(N_FFT) / N_FFT))
    angle = 2.0 * np.pi * np.outer(k, n) / N_FFT
    w_fwd_r = np.cos(angle) * win
    w_fwd_i = -np.sin(angle) * win
    inv_win = win / N_FFT
    w_bwd_r = np.cos(angle) * inv_win
    w_bwd_i = np.sin(angle) * inv_win
    M = w_bwd_r.T @ w_fwd_r - w_bwd_i.T @ w_fwd_i          # [800, 800] f64

    # Band kernels K_d[r, s] = sum_i M[i*C+r, (i+d)*C+s], d in [-3..3]
    # Kfull[r, q] with q = (d+3)*C + s -> out[t] = sum_{j'} K_{j'} @ Pch[t+j']
    Kfull = np.zeros((C, 7 * C))
    for d in range(-3, 4):
        for i in range(4):
            j = i + d
            if 0 <= j <= 3:
                Kfull[:, (d + 3) * C:(d + 4) * C] += M[i * C:(i + 1) * C, j * C:(j + 1) * C]

    # lhsT blocks for the interior matmuls: b = (j'*2 + h)*2 + ho
    # KTT[s', b*H + r'] = Kfull[ho*H + r', j'*C + h*H + s']
    KTT = np.zeros((H, 28 * H), dtype=np.float32)
    for jp in range(7):
        for h in range(2):
            for ho in range(2):
                b = (jp * 2 + h) * 2 + ho
                blk = Kfull[ho * H:(ho + 1) * H, jp * C + h * H: jp * C + h * H + H]
                KTT[:, b * H:(b + 1) * H] = blk.T.astype(np.float32)

    # Edge corrections (pre-negated so they accumulate subtractively):
    # left  (out chunk t=0):    minus sum_{j=1..3} M[3C:4C, jC:(j+1)C] @ xp_chunk[j-1]
    # right (out chunk t=1199): minus sum_{j=0..2} M[0:C,  jC:(j+1)C] @ xp_chunk[1201+j]
    ET = np.zeros((H, 24 * H), dtype=np.float32)
    for side in range(2):
        for ho in range(2):
            for jj in range(3):
                for h in range(2):
                    if side == 0:
                        blk = M[3 * C + ho * H: 3 * C + (ho + 1) * H,
                                (jj + 1) * C + h * H: (jj + 1) * C + h * H + H]
                    else:
                        blk = M[ho * H:(ho + 1) * H,
                                jj * C + h * H: jj * C + h * H + H]
                    eb = ((side * 2 + ho) * 3 + jj) * 2 + h
                    ET[:, eb * H:(eb + 1) * H] = (-blk.T).astype(np.float32)

    _cache["ktt"], _cache["et"] = KTT, ET
    return KTT, ET


def _build_nc():
    if "nc" in _cache:
        return _cache["nc"]
    import concourse.mybir as mybir
    import concourse.tile as tile
    from concourse import bacc

    f32 = mybir.dt.float32
    f32r = mybir.dt.float32r

    nc = bacc.Bacc("TRN2", target_bir_lowering=False, debug=False,
                   num_devices=N_CORES)
    ph_d = nc.dram_tensor("ph", [2, H, SPC, NCH_P], f32r, kind="ExternalInput").ap()
    ktt_d = nc.dram_tensor("ktt", [H, 28 * H], f32r, kind="ExternalInput").ap()
    et_d = nc.dram_tensor("et", [H, 24 * H], f32r, kind="ExternalInput").ap()
    out_d = nc.dram_tensor("out", [SPC, 2, H, NOUT], f32, kind="ExternalOutput").ap()

    with tile.TileContext(nc) as tc:
        with (
            tc.tile_pool(name="weights", bufs=1) as wpool,
            tc.tile_pool(name="data", bufs=1) as dpool,
            tc.tile_pool(name="outp", bufs=1) as opool,
            tc.tile_pool(name="pint", bufs=3, space="PSUM") as pint,
            tc.tile_pool(name="pedge", bufs=1, space="PSUM") as pedge,
        ):
            ktt = wpool.tile([H, 28 * H], f32r)
            et = wpool.tile([H, 24 * H], f32r)
            nc.sync.dma_start(ktt[:], ktt_d[:])
            nc.sync.dma_start(et[:], et_d[:])

            phs = [dpool.tile([H, SPC, NCH_P], f32r, name=f"ph{h}") for h in range(2)]
            for h in range(2):
                nc.sync.dma_start(phs[h][:], ph_d[h])

            outs = [[opool.tile([H, NOUT], f32, name=f"out{ss}_{ho}")
                     for ho in range(2)] for ss in range(SPC)]

            # Edge-correction matmuls: 4 psum tiles [H, SPC], each accumulates
            # 6 small matmuls (both samples per matmul via the ss free dim).
            pes = [[pedge.tile([H, SPC], f32, name=f"pe{side}_{ho}")
                    for ho in range(2)] for side in range(2)]
            for side in range(2):
                for ho in range(2):
                    for jj in range(3):
                        for h in range(2):
                            eb = ((side * 2 + ho) * 3 + jj) * 2 + h
                            cp = (1 + jj) if side == 0 else (1202 + jj)
                            nc.tensor.matmul(
                                pes[side][ho][:],
                                et[:, eb * H:(eb + 1) * H],
                                phs[h][:, :, cp],
                                start=(jj == 0 and h == 0),
                                stop=(jj == 2 and h == 1),
                            )

            # Interior: per (sample, tile, out-half) accumulate 14 matmuls.
            for ss in range(SPC):
                for tidx in range(NT):
                    t0 = tidx * NTILE
                    for ho in range(2):
                        ps = pint.tile([H, NTILE], f32, tag="ps")
                        for jp in range(7):
                            for h in range(2):
                                bidx = (jp * 2 + h) * 2 + ho
                                nc.tensor.matmul(
                                    ps[:],
                                    ktt[:, bidx * H:(bidx + 1) * H],
                                    phs[h][:, ss, t0 + jp: t0 + jp + NTILE],
                                    start=(jp == 0 and h == 0),
                                    stop=(jp == 6 and h == 1),
                                )
                        nc.vector.tensor_copy(outs[ss][ho][:, t0:t0 + NTILE], ps[:])

            # Apply edge corrections and store.
            for ss in range(SPC):
                for ho in range(2):
                    nc.vector.tensor_add(
                        outs[ss][ho][:, 0:1], outs[ss][ho][:, 0:1],
                        pes[0][ho][:, ss:ss + 1])
                    nc.vector.tensor_add(
                        outs[ss][ho][:, NOUT - 1:NOUT], outs[ss][ho][:, NOUT - 1:NOUT],
                        pes[1][ho][:, ss:ss + 1])
                    nc.sync.dma_start(out_d[ss, ho], outs[ss][ho][:])

    nc.compile()
    _cache["nc"] = nc
    return nc


last_results = None  # BassKernelResults of the most recent run (for test harness)


def kernel(x, w_fwd_r=None, w_fwd_i=None, w_bwd_r=None, w_bwd_i=None):
    global last_results
    from concourse.bass_utils import run_bass_kernel_spmd

    x = np.asarray(x, dtype=np.float32)
    assert x.shape == (B, T), x.shape
    KTT, ET = _host_weights()
    nc = _build_nc()

    # Host marshalling: edge-pad, zero-chunk-pad, split chunks into halves,
    # transpose so the contraction index s' (0..99) is the partition dim.
    xp = np.pad(x, ((0, 0), (PAD, PAD)), mode="edge")       # [B, 240800]
    P = np.pad(xp, ((0, 0), (C, C)))                        # [B, 241200]
    # ph[core][h, s', ss, c'] = P[2*core+ss, c'*200 + h*100 + s']
    P4 = P.reshape(B, NCH_P, 2, H)                          # (b, c', h, s')
    in_maps = []
    for core in range(N_CORES):
        blk = P4[core * SPC:(core + 1) * SPC]               # (ss, c', h, s')
        ph = np.ascontiguousarray(blk.transpose(2, 3, 0, 1))  # (h, s', ss, c')
        in_maps.append({"ph": ph, "ktt": KTT, "et": ET})

    res = run_bass_kernel_spmd(nc, in_maps, core_ids=list(range(N_CORES)))
    last_results = res

    y = np.empty((B, T), dtype=np.float32)
    for core in range(N_CORES):
        od = res.results[core]["out"]                       # [SPC, 2, H, NOUT]
        for ss in range(SPC):
            # y[t*200 + ho*100 + r'] = od[ss, ho, r', t]
            y[core * SPC + ss] = od[ss].transpose(2, 0, 1).reshape(T)
    return y


# revision 3
# speedup vs baseline: 1.2336x; 1.2336x over previous
"""Trainium2 Bass kernel for CustomSTFT (STFT -> mag/phase -> ISTFT roundtrip).

Key insight: magnitude*cos(atan2(i,r)) == r and magnitude*sin(atan2(i,r)) == i
(up to a 1e-14 epsilon inside the sqrt), so the whole pipeline is linear in x:

    y = OLA( frames @ M.T ),   M = w_bwd_r.T @ w_fwd_r - w_bwd_i.T @ w_fwd_i

With hop=200 and n_fft=800, collapsing the frame/OLA structure per 200-sample
chunk gives a 7-band polyphase filter: out_chunk[c] = sum_{d=-3..3} K_d @ in_chunk[c+d]
plus two boundary corrections (first/last output chunk miss one phantom frame).

Sharding: batch 16 across 8 cores (2 samples each). Matmuls in fp16 (weights
and data; f32 PSUM accumulation) — measured ~5e-4 scale-relative absmax.
"""

import numpy as np

N_CORES = 8
B, T = 16, 240000
SPC = B // N_CORES          # samples per core
N_FFT, HOP, FREQ, PAD = 800, 200, 401, 400
C = HOP                     # chunk size 200
H = C // 2                  # 100: SBUF partition dim (contraction split)
NCH_XP = (T + 2 * PAD) // C     # 1204 chunks in edge-padded signal
NCH_P = NCH_XP + 2              # 1206 incl. one zero chunk each side
NOUT = T // C                   # 1200 output chunks per sample
NTILE = 400                     # output chunks per matmul (psum free dim)
NT = NOUT // NTILE              # 3 interior tiles per sample
PH_SPLIT = NTILE + 6            # ph DMA piece boundary: cols [0,406) / [406,1206)

_cache = {}


def _host_weights():
    """Build the device weight tensors (fp16) from first principles."""
    if "ktt" in _cache:
        return _cache["ktt"], _cache["et"]
    n = np.arange(N_FFT)
    k = np.arange(FREQ)
    win = 0.5 * (1.0 - np.cos(2.0 * np.pi * np.arange(N_FFT) / N_FFT))
    angle = 2.0 * np.pi * np.outer(k, n) / N_FFT
    w_fwd_r = np.cos(angle) * win
    w_fwd_i = -np.sin(angle) * win
    inv_win = win / N_FFT
    w_bwd_r = np.cos(angle) * inv_win
    w_bwd_i = np.sin(angle) * inv_win
    M = w_bwd_r.T @ w_fwd_r - w_bwd_i.T @ w_fwd_i          # [800, 800] f64

    # Band kernels K_d[r, s] = sum_i M[i*C+r, (i+d)*C+s], d in [-3..3]
    # Kfull[r, q] with q = (d+3)*C + s -> out[t] = sum_{j'} K_{j'} @ Pch[t+j']
    Kfull = np.zeros((C, 7 * C))
    for d in range(-3, 4):
        for i in range(4):
            j = i + d
            if 0 <= j <= 3:
                Kfull[:, (d + 3) * C:(d + 4) * C] += M[i * C:(i + 1) * C, j * C:(j + 1) * C]

    # lhsT blocks for the interior matmuls, ho-major so the first psum groups
    # only gate on the first half of the weight DMA: b = ho*14 + jp*2 + h
    # KTT[s', b*H + r'] = Kfull[ho*H + r', jp*C + h*H + s']
    KTT = np.zeros((H, 28 * H), dtype=np.float16)
    for ho in range(2):
        for jp in range(7):
            for h in range(2):
                b = ho * 14 + jp * 2 + h
                blk = Kfull[ho * H:(ho + 1) * H, jp * C + h * H: jp * C + h * H + H]
                KTT[:, b * H:(b + 1) * H] = blk.T.astype(np.float16)

    # Edge corrections (pre-negated so they accumulate subtractively):
    # left  (out chunk t=0):    minus sum_{j=1..3} M[3C:4C, jC:(j+1)C] @ xp_chunk[j-1]
    # right (out chunk t=1199): minus sum_{j=0..2} M[0:C,  jC:(j+1)C] @ xp_chunk[1201+j]
    ET = np.zeros((H, 24 * H), dtype=np.float16)
    for side in range(2):
        for ho in range(2):
            for jj in range(3):
                for h in range(2):
                    if side == 0:
                        blk = M[3 * C + ho * H: 3 * C + (ho + 1) * H,
                                (jj + 1) * C + h * H: (jj + 1) * C + h * H + H]
                    else:
                        blk = M[ho * H:(ho + 1) * H,
                                jj * C + h * H: jj * C + h * H + H]
                    eb = ((side * 2 + ho) * 3 + jj) * 2 + h
                    ET[:, eb * H:(eb + 1) * H] = (-blk.T).astype(np.float16)

    _cache["ktt"], _cache["et"] = KTT, ET
    return KTT, ET


def _build_nc():
    if "nc" in _cache:
        return _cache["nc"]
    import concourse.mybir as mybir
    import concourse.tile as tile
    from concourse import bacc

    f32 = mybir.dt.float32
    f16 = mybir.dt.float16

    nc = bacc.Bacc("TRN2", target_bir_lowering=False, debug=False,
                   num_devices=N_CORES)
    ph_d = nc.dram_tensor("ph", [2, H, SPC, NCH_P], f16, kind="ExternalInput").ap()
    ktt_d = nc.dram_tensor("ktt", [H, 28 * H], f16, kind="ExternalInput").ap()
    et_d = nc.dram_tensor("et", [H, 24 * H], f16, kind="ExternalInput").ap()
    out_d = nc.dram_tensor("out", [SPC, 2, H, NOUT], f32, kind="ExternalOutput").ap()

    with tile.TileContext(nc) as tc:
        with (
            tc.tile_pool(name="weights", bufs=1) as wpool,
            tc.tile_pool(name="data", bufs=1) as dpool,
            tc.tile_pool(name="outp", bufs=1) as opool,
            tc.tile_pool(name="pint", bufs=4, space="PSUM") as pint,
            tc.tile_pool(name="pedge", bufs=1, space="PSUM") as pedge,
        ):
            ktt = wpool.tile([H, 28 * H], f16)
            et = wpool.tile([H, 24 * H], f16)
            phs = [dpool.tile([H, SPC, NCH_P], f16, name=f"ph{h}") for h in range(2)]
            outs = [[opool.tile([H, NOUT], f32, name=f"out{ss}_{ho}")
                     for ho in range(2)] for ss in range(SPC)]

            # --- input DMA, pieced for early compute start; triggers spread
            # across the three DMA-capable lanes (sync/scalar HWDGE, gpsimd
            # SWDGE); each trigger blocks its lane ~0.9us ---
            nc.scalar.dma_start(ktt[:, 0:14 * H], ktt_d[:, 0:14 * H])
            for h in range(2):
                nc.sync.dma_start(phs[h][:, 0, 0:PH_SPLIT],
                                  ph_d[h][:, 0, 0:PH_SPLIT])
            nc.gpsimd.dma_start(ktt[:, 14 * H:28 * H], ktt_d[:, 14 * H:28 * H])
            for h in range(2):
                nc.sync.dma_start(phs[h][:, 0, PH_SPLIT:NCH_P],
                                  ph_d[h][:, 0, PH_SPLIT:NCH_P])
            for h in range(2):
                nc.scalar.dma_start(phs[h][:, 1, 0:PH_SPLIT],
                                    ph_d[h][:, 1, 0:PH_SPLIT])
            for h in range(2):
                nc.scalar.dma_start(phs[h][:, 1, PH_SPLIT:NCH_P],
                                    ph_d[h][:, 1, PH_SPLIT:NCH_P])
            nc.gpsimd.dma_start(et[:], et_d[:])

            # --- interior: per (sample, tile, out-half) accumulate 14 matmuls ---
            for ss in range(SPC):
                for tidx in range(NT):
                    t0 = tidx * NTILE
                    for ho in range(2):
                        ps = pint.tile([H, NTILE], f32, tag="ps")
                        for jp in range(7):
                            for h in range(2):
                                bidx = ho * 14 + jp * 2 + h
                                nc.tensor.matmul(
                                    ps[:],
                                    ktt[:, bidx * H:(bidx + 1) * H],
                                    phs[h][:, ss, t0 + jp: t0 + jp + NTILE],
                                    start=(jp == 0 and h == 0),
                                    stop=(jp == 6 and h == 1),
                                )
                        nc.vector.tensor_copy(outs[ss][ho][:, t0:t0 + NTILE], ps[:])

            # --- edge-correction matmuls: 4 psum tiles [H, SPC], each
            # accumulating 6 blocks (both samples per matmul via ss stride) ---
            pes = [[pedge.tile([H, SPC], f32, name=f"pe{side}_{ho}")
                    for ho in range(2)] for side in range(2)]
            for side in range(2):
                for ho in range(2):
                    for jj in range(3):
                        for h in range(2):
                            eb = ((side * 2 + ho) * 3 + jj) * 2 + h
                            cp = (1 + jj) if side == 0 else (1202 + jj)
                            nc.tensor.matmul(
                                pes[side][ho][:],
                                et[:, eb * H:(eb + 1) * H],
                                phs[h][:, :, cp],
                                start=(jj == 0 and h == 0),
                                stop=(jj == 2 and h == 1),
                            )

            # --- apply edge corrections; store output in two pieces ---
            for ss in range(SPC):
                for ho in range(2):
                    nc.vector.tensor_add(
                        outs[ss][ho][:, 0:1], outs[ss][ho][:, 0:1],
                        pes[0][ho][:, ss:ss + 1])
                    nc.vector.tensor_add(
                        outs[ss][ho][:, NOUT - 1:NOUT], outs[ss][ho][:, NOUT - 1:NOUT],
                        pes[1][ho][:, ss:ss + 1])
                    nc.gpsimd.dma_start(out_d[ss, ho][:, 0:2 * NTILE],
                                        outs[ss][ho][:, 0:2 * NTILE])
                    nc.gpsimd.dma_start(out_d[ss, ho][:, 2 * NTILE:NOUT],
                                        outs[ss][ho][:, 2 * NTILE:NOUT])

    nc.compile()
    _cache["nc"] = nc
    return nc


last_results = None  # BassKernelResults of the most recent run (for test harness)


def kernel(x, w_fwd_r=None, w_fwd_i=None, w_bwd_r=None, w_bwd_i=None):
    global last_results
    from concourse.bass_utils import run_bass_kernel_spmd

    x = np.asarray(x, dtype=np.float32)
    assert x.shape == (B, T), x.shape
    KTT, ET = _host_weights()
    nc = _build_nc()

    # Host marshalling: edge-pad, zero-chunk-pad, split chunks into halves,
    # transpose so the contraction index s' (0..99) is the partition dim.
    xp = np.pad(x, ((0, 0), (PAD, PAD)), mode="edge")       # [B, 240800]
    P = np.pad(xp, ((0, 0), (C, C))).astype(np.float16)     # [B, 241200]
    # ph[core][h, s', ss, c'] = P[2*core+ss, c'*200 + h*100 + s']
    P4 = P.reshape(B, NCH_P, 2, H)                          # (b, c', h, s')
    in_maps = []
    for core in range(N_CORES):
        blk = P4[core * SPC:(core + 1) * SPC]               # (ss, c', h, s')
        ph = np.ascontiguousarray(blk.transpose(2, 3, 0, 1))  # (h, s', ss, c')
        in_maps.append({"ph": ph, "ktt": KTT, "et": ET})

    res = run_bass_kernel_spmd(nc, in_maps, core_ids=list(range(N_CORES)))
    last_results = res

    y = np.empty((B, T), dtype=np.float32)
    for core in range(N_CORES):
        od = res.results[core]["out"]                       # [SPC, 2, H, NOUT]
        for ss in range(SPC):
            # y[t*200 + ho*100 + r'] = od[ss, ho, r', t]
            y[core * SPC + ss] = od[ss].transpose(2, 0, 1).reshape(T)
    return y


# revision 5
# speedup vs baseline: 1.4571x; 1.1811x over previous
"""Trainium2 Bass kernel for CustomSTFT (STFT -> mag/phase -> ISTFT roundtrip).

Key insight: magnitude*cos(atan2(i,r)) == r and magnitude*sin(atan2(i,r)) == i
(up to a 1e-14 epsilon inside the sqrt), so the whole pipeline is linear in x:

    y = OLA( frames @ M.T ),   M = w_bwd_r.T @ w_fwd_r - w_bwd_i.T @ w_fwd_i

With hop=200 and n_fft=800, collapsing the frame/OLA structure per 200-sample
chunk gives a 7-band polyphase filter: out_chunk[c] = sum_{d=-3..3} K_d @ in_chunk[c+d]
plus two boundary corrections (first/last output chunk miss one phantom frame).

Sharding: batch 16 across 8 cores (2 samples each). Matmuls in fp16 (weights
and data; f32 PSUM accumulation) — measured ~5e-4 scale-relative absmax.
"""

import numpy as np

N_CORES = 8
B, T = 16, 240000
SPC = B // N_CORES          # samples per core
N_FFT, HOP, FREQ, PAD = 800, 200, 401, 400
C = HOP                     # chunk size 200
H = C // 2                  # 100: SBUF partition dim (contraction split)
NCH_XP = (T + 2 * PAD) // C     # 1204 chunks in edge-padded signal
NCH_P = NCH_XP + 2              # 1206 incl. one zero chunk each side
NOUT = T // C                   # 1200 output chunks per sample
NTILE = 400                     # output chunks per matmul (psum free dim)
NT = NOUT // NTILE              # 3 interior tiles per sample
PH_SPLIT = NTILE + 6            # ph DMA piece boundary: cols [0,406) / [406,1206)

_cache = {}


def _host_weights():
    """Build the device weight tensors (fp16) from first principles."""
    if "ktt" in _cache:
        return _cache["ktt"], _cache["et"]
    n = np.arange(N_FFT)
    k = np.arange(FREQ)
    win = 0.5 * (1.0 - np.cos(2.0 * np.pi * np.arange(N_FFT) / N_FFT))
    angle = 2.0 * np.pi * np.outer(k, n) / N_FFT
    w_fwd_r = np.cos(angle) * win
    w_fwd_i = -np.sin(angle) * win
    inv_win = win / N_FFT
    w_bwd_r = np.cos(angle) * inv_win
    w_bwd_i = np.sin(angle) * inv_win
    M = w_bwd_r.T @ w_fwd_r - w_bwd_i.T @ w_fwd_i          # [800, 800] f64

    # Band kernels K_d[r, s] = sum_i M[i*C+r, (i+d)*C+s], d in [-3..3]
    # Kfull[r, q] with q = (d+3)*C + s -> out[t] = sum_{j'} K_{j'} @ Pch[t+j']
    Kfull = np.zeros((C, 7 * C))
    for d in range(-3, 4):
        for i in range(4):
            j = i + d
            if 0 <= j <= 3:
                Kfull[:, (d + 3) * C:(d + 4) * C] += M[i * C:(i + 1) * C, j * C:(j + 1) * C]

    # lhsT blocks for the interior matmuls, ho-major so the first psum groups
    # only gate on the first half of the weight DMA: b = ho*14 + jp*2 + h
    # KTT[s', b*H + r'] = Kfull[ho*H + r', jp*C + h*H + s']
    KTT = np.zeros((H, 28 * H), dtype=np.float16)
    for ho in range(2):
        for jp in range(7):
            for h in range(2):
                b = ho * 14 + jp * 2 + h
                blk = Kfull[ho * H:(ho + 1) * H, jp * C + h * H: jp * C + h * H + H]
                KTT[:, b * H:(b + 1) * H] = blk.T.astype(np.float16)

    # Edge corrections (pre-negated so they accumulate subtractively):
    # left  (out chunk t=0):    minus sum_{j=1..3} M[3C:4C, jC:(j+1)C] @ xp_chunk[j-1]
    # right (out chunk t=1199): minus sum_{j=0..2} M[0:C,  jC:(j+1)C] @ xp_chunk[1201+j]
    ET = np.zeros((H, 24 * H), dtype=np.float16)
    for side in range(2):
        for ho in range(2):
            for jj in range(3):
                for h in range(2):
                    if side == 0:
                        blk = M[3 * C + ho * H: 3 * C + (ho + 1) * H,
                                (jj + 1) * C + h * H: (jj + 1) * C + h * H + H]
                    else:
                        blk = M[ho * H:(ho + 1) * H,
                                jj * C + h * H: jj * C + h * H + H]
                    eb = ((side * 2 + ho) * 3 + jj) * 2 + h
                    ET[:, eb * H:(eb + 1) * H] = (-blk.T).astype(np.float16)

    _cache["ktt"], _cache["et"] = KTT, ET
    return KTT, ET


def _build_nc():
    if "nc" in _cache:
        return _cache["nc"]
    import concourse.mybir as mybir
    import concourse.tile as tile
    from concourse import bacc

    f32 = mybir.dt.float32
    f16 = mybir.dt.float16

    nc = bacc.Bacc("TRN2", target_bir_lowering=False, debug=False,
                   num_devices=N_CORES)
    ph_d = nc.dram_tensor("ph", [2, H, SPC, NCH_P], f16, kind="ExternalInput").ap()
    ktt_d = nc.dram_tensor("ktt", [H, 28 * H], f16, kind="ExternalInput").ap()
    et_d = nc.dram_tensor("et", [H, 24 * H], f16, kind="ExternalInput").ap()
    out_d = nc.dram_tensor("out", [SPC, 2, H, NOUT], f32, kind="ExternalOutput").ap()

    with tile.TileContext(nc) as tc:
        with (
            tc.tile_pool(name="weights", bufs=1) as wpool,
            tc.tile_pool(name="data", bufs=1) as dpool,
            tc.tile_pool(name="outp", bufs=1) as opool,
            tc.tile_pool(name="pint", bufs=4, space="PSUM") as pint,
            tc.tile_pool(name="pedge", bufs=1, space="PSUM") as pedge,
        ):
            ktt = wpool.tile([H, 28 * H], f16)
            et = wpool.tile([H, 24 * H], f16)
            phs = [dpool.tile([H, SPC, NCH_P], f16, name=f"ph{h}") for h in range(2)]
            outs = [[opool.tile([H, NOUT], f32, name=f"out{ss}_{ho}")
                     for ho in range(2)] for ss in range(SPC)]

            # --- input DMA, pieced for early compute start; triggers spread
            # across the two HWDGE lanes (sync, scalar); each trigger blocks
            # its lane ~0.9us. gpsimd SWDGE is slow — don't use it. ---
            for h in range(2):
                nc.sync.dma_start(phs[h][:, 0, 0:PH_SPLIT],
                                  ph_d[h][:, 0, 0:PH_SPLIT])
            nc.scalar.dma_start(ktt[:, 0:14 * H], ktt_d[:, 0:14 * H])
            for h in range(2):
                nc.scalar.dma_start(phs[h][:, 1, 0:PH_SPLIT],
                                    ph_d[h][:, 1, 0:PH_SPLIT])
            nc.sync.dma_start(et[:], et_d[:])
            for h in range(2):
                nc.sync.dma_start(phs[h][:, 0, PH_SPLIT:NCH_P],
                                  ph_d[h][:, 0, PH_SPLIT:NCH_P])
            nc.scalar.dma_start(ktt[:, 14 * H:28 * H], ktt_d[:, 14 * H:28 * H])
            for h in range(2):
                nc.scalar.dma_start(phs[h][:, 1, PH_SPLIT:NCH_P],
                                    ph_d[h][:, 1, PH_SPLIT:NCH_P])

            # --- edge-correction matmuls first (cheap; unblock the output
            # stores early): 4 psum tiles [H, SPC], each accumulating 6
            # blocks (both samples per matmul via the ss stride) ---
            pes = [[pedge.tile([H, SPC], f32, name=f"pe{side}_{ho}")
                    for ho in range(2)] for side in range(2)]
            for side in range(2):
                for ho in range(2):
                    for jj in range(3):
                        for h in range(2):
                            eb = ((side * 2 + ho) * 3 + jj) * 2 + h
                            cp = (1 + jj) if side == 0 else (1202 + jj)
                            nc.tensor.matmul(
                                pes[side][ho][:],
                                et[:, eb * H:(eb + 1) * H],
                                phs[h][:, :, cp],
                                start=(jj == 0 and h == 0),
                                stop=(jj == 2 and h == 1),
                            )

            # --- interior: per (sample, tile, out-half) accumulate 14 matmuls ---
            for ss in range(SPC):
                for tidx in range(NT):
                    t0 = tidx * NTILE
                    for ho in range(2):
                        ps = pint.tile([H, NTILE], f32, tag="ps")
                        for jp in range(7):
                            for h in range(2):
                                bidx = ho * 14 + jp * 2 + h
                                nc.tensor.matmul(
                                    ps[:],
                                    ktt[:, bidx * H:(bidx + 1) * H],
                                    phs[h][:, ss, t0 + jp: t0 + jp + NTILE],
                                    start=(jp == 0 and h == 0),
                                    stop=(jp == 6 and h == 1),
                                )
                        nc.vector.tensor_copy(outs[ss][ho][:, t0:t0 + NTILE], ps[:])

            # --- apply edge corrections; store output in two pieces ---
            for ss in range(SPC):
                eng = (nc.sync, nc.scalar)[ss]
                for ho in range(2):
                    nc.vector.tensor_add(
                        outs[ss][ho][:, 0:1], outs[ss][ho][:, 0:1],
                        pes[0][ho][:, ss:ss + 1])
                    nc.vector.tensor_add(
                        outs[ss][ho][:, NOUT - 1:NOUT], outs[ss][ho][:, NOUT - 1:NOUT],
                        pes[1][ho][:, ss:ss + 1])
                    eng.dma_start(out_d[ss, ho][:, 0:2 * NTILE],
                                  outs[ss][ho][:, 0:2 * NTILE])
                    eng.dma_start(out_d[ss, ho][:, 2 * NTILE:NOUT],
                                  outs[ss][ho][:, 2 * NTILE:NOUT])

    nc.compile()
    _cache["nc"] = nc
    return nc


last_results = None  # BassKernelResults of the most recent run (for test harness)


def kernel(x, w_fwd_r=None, w_fwd_i=None, w_bwd_r=None, w_bwd_i=None):
    global last_results
    from concourse.bass_utils import run_bass_kernel_spmd

    x = np.asarray(x, dtype=np.float32)
    assert x.shape == (B, T), x.shape
    KTT, ET = _host_weights()
    nc = _build_nc()

    # Host marshalling: edge-pad, zero-chunk-pad, split chunks into halves,
    # transpose so the contraction index s' (0..99) is the partition dim.
    xp = np.pad(x, ((0, 0), (PAD, PAD)), mode="edge")       # [B, 240800]
    P = np.pad(xp, ((0, 0), (C, C))).astype(np.float16)     # [B, 241200]
    # ph[core][h, s', ss, c'] = P[2*core+ss, c'*200 + h*100 + s']
    P4 = P.reshape(B, NCH_P, 2, H)                          # (b, c', h, s')
    in_maps = []
    for core in range(N_CORES):
        blk = P4[core * SPC:(core + 1) * SPC]               # (ss, c', h, s')
        ph = np.ascontiguousarray(blk.transpose(2, 3, 0, 1))  # (h, s', ss, c')
        in_maps.append({"ph": ph, "ktt": KTT, "et": ET})

    res = run_bass_kernel_spmd(nc, in_maps, core_ids=list(range(N_CORES)))
    last_results = res

    y = np.empty((B, T), dtype=np.float32)
    for core in range(N_CORES):
        od = res.results[core]["out"]                       # [SPC, 2, H, NOUT]
        for ss in range(SPC):
            # y[t*200 + ho*100 + r'] = od[ss, ho, r', t]
            y[core * SPC + ss] = od[ss].transpose(2, 0, 1).reshape(T)
    return y


# revision 6
# speedup vs baseline: 1.4692x; 1.0083x over previous
"""Trainium2 Bass kernel for CustomSTFT (STFT -> mag/phase -> ISTFT roundtrip).

Key insight: magnitude*cos(atan2(i,r)) == r and magnitude*sin(atan2(i,r)) == i
(up to a 1e-14 epsilon inside the sqrt), so the whole pipeline is linear in x:

    y = OLA( frames @ M.T ),   M = w_bwd_r.T @ w_fwd_r - w_bwd_i.T @ w_fwd_i

With hop=200 and n_fft=800, collapsing the frame/OLA structure per 200-sample
chunk gives a 7-band polyphase filter: out_chunk[c] = sum_{d=-3..3} K_d @ in_chunk[c+d]
plus two boundary corrections (first/last output chunk miss one phantom frame).

Sharding: batch 16 across 8 cores (2 samples each). Matmuls in fp16 (weights
and data; f32 PSUM accumulation) — measured ~5e-4 scale-relative absmax.
"""

import numpy as np

N_CORES = 8
B, T = 16, 240000
SPC = B // N_CORES          # samples per core
N_FFT, HOP, FREQ, PAD = 800, 200, 401, 400
C = HOP                     # chunk size 200
H = C // 2                  # 100: SBUF partition dim (contraction split)
NCH_XP = (T + 2 * PAD) // C     # 1204 chunks in edge-padded signal
NCH_P = NCH_XP + 2              # 1206 incl. one zero chunk each side
NOUT = T // C                   # 1200 output chunks per sample
NTILE = 400                     # output chunks per matmul (psum free dim)
NT = NOUT // NTILE              # 3 interior tiles per sample
PH_SPLIT = NTILE + 6            # ph DMA piece boundary: cols [0,406) / [406,1206)
N_WARM = 12                     # dummy matmuls to warm the PE clock (HAM)

_cache = {}


def _host_weights():
    """Build the device weight tensors (fp16) from first principles."""
    if "ktt" in _cache:
        return _cache["ktt"], _cache["et"]
    n = np.arange(N_FFT)
    k = np.arange(FREQ)
    win = 0.5 * (1.0 - np.cos(2.0 * np.pi * np.arange(N_FFT) / N_FFT))
    angle = 2.0 * np.pi * np.outer(k, n) / N_FFT
    w_fwd_r = np.cos(angle) * win
    w_fwd_i = -np.sin(angle) * win
    inv_win = win / N_FFT
    w_bwd_r = np.cos(angle) * inv_win
    w_bwd_i = np.sin(angle) * inv_win
    M = w_bwd_r.T @ w_fwd_r - w_bwd_i.T @ w_fwd_i          # [800, 800] f64

    # Band kernels K_d[r, s] = sum_i M[i*C+r, (i+d)*C+s], d in [-3..3]
    # Kfull[r, q] with q = (d+3)*C + s -> out[t] = sum_{j'} K_{j'} @ Pch[t+j']
    Kfull = np.zeros((C, 7 * C))
    for d in range(-3, 4):
        for i in range(4):
            j = i + d
            if 0 <= j <= 3:
                Kfull[:, (d + 3) * C:(d + 4) * C] += M[i * C:(i + 1) * C, j * C:(j + 1) * C]

    # lhsT blocks for the interior matmuls, ho-major so the first psum groups
    # only gate on the first half of the weight DMA: b = ho*14 + jp*2 + h
    # KTT[s', b*H + r'] = Kfull[ho*H + r', jp*C + h*H + s']
    KTT = np.zeros((H, 28 * H), dtype=np.float16)
    for ho in range(2):
        for jp in range(7):
            for h in range(2):
                b = ho * 14 + jp * 2 + h
                blk = Kfull[ho * H:(ho + 1) * H, jp * C + h * H: jp * C + h * H + H]
                KTT[:, b * H:(b + 1) * H] = blk.T.astype(np.float16)

    # Edge corrections (pre-negated so they accumulate subtractively):
    # left  (out chunk t=0):    minus sum_{j=1..3} M[3C:4C, jC:(j+1)C] @ xp_chunk[j-1]
    # right (out chunk t=1199): minus sum_{j=0..2} M[0:C,  jC:(j+1)C] @ xp_chunk[1201+j]
    ET = np.zeros((H, 24 * H), dtype=np.float16)
    for side in range(2):
        for ho in range(2):
            for jj in range(3):
                for h in range(2):
                    if side == 0:
                        blk = M[3 * C + ho * H: 3 * C + (ho + 1) * H,
                                (jj + 1) * C + h * H: (jj + 1) * C + h * H + H]
                    else:
                        blk = M[ho * H:(ho + 1) * H,
                                jj * C + h * H: jj * C + h * H + H]
                    eb = ((side * 2 + ho) * 3 + jj) * 2 + h
                    ET[:, eb * H:(eb + 1) * H] = (-blk.T).astype(np.float16)

    _cache["ktt"], _cache["et"] = KTT, ET
    return KTT, ET


def _build_nc():
    if "nc" in _cache:
        return _cache["nc"]
    import concourse.mybir as mybir
    import concourse.tile as tile
    from concourse import bacc

    f32 = mybir.dt.float32
    f16 = mybir.dt.float16

    nc = bacc.Bacc("TRN2", target_bir_lowering=False, debug=False,
                   num_devices=N_CORES)
    ph_d = nc.dram_tensor("ph", [2, H, SPC, NCH_P], f16, kind="ExternalInput").ap()
    ktt_d = nc.dram_tensor("ktt", [H, 28 * H], f16, kind="ExternalInput").ap()
    et_d = nc.dram_tensor("et", [H, 24 * H], f16, kind="ExternalInput").ap()
    out_d = nc.dram_tensor("out", [SPC, 2, H, NOUT], f32, kind="ExternalOutput").ap()

    with tile.TileContext(nc) as tc:
        with (
            tc.tile_pool(name="weights", bufs=1) as wpool,
            tc.tile_pool(name="data", bufs=1) as dpool,
            tc.tile_pool(name="outp", bufs=1) as opool,
            tc.tile_pool(name="pint", bufs=4, space="PSUM") as pint,
            tc.tile_pool(name="pedge", bufs=1, space="PSUM") as pedge,
        ):
            ktt = wpool.tile([H, 28 * H], f16)
            et = wpool.tile([H, 24 * H], f16)
            warm = wpool.tile([H, NTILE], f16)
            phs = [dpool.tile([H, SPC, NCH_P], f16, name=f"ph{h}") for h in range(2)]
            outs = [[opool.tile([H, NOUT], f32, name=f"out{ss}_{ho}")
                     for ho in range(2)] for ss in range(SPC)]

            nc.gpsimd.memset(warm[:], 0.0)

            # --- input DMA, pieced for early compute start; triggers spread
            # across the two HWDGE lanes (sync, scalar); each trigger blocks
            # its lane ~0.9us. gpsimd SWDGE is slow — don't use it. ---
            for h in range(2):
                nc.sync.dma_start(phs[h][:, 0, 0:PH_SPLIT],
                                  ph_d[h][:, 0, 0:PH_SPLIT])
            nc.scalar.dma_start(ktt[:, 0:14 * H], ktt_d[:, 0:14 * H])
            for h in range(2):
                nc.scalar.dma_start(phs[h][:, 1, 0:PH_SPLIT],
                                    ph_d[h][:, 1, 0:PH_SPLIT])
            nc.sync.dma_start(et[:], et_d[:])
            for h in range(2):
                nc.sync.dma_start(phs[h][:, 0, PH_SPLIT:NCH_P],
                                  ph_d[h][:, 0, PH_SPLIT:NCH_P])
            nc.scalar.dma_start(ktt[:, 14 * H:28 * H], ktt_d[:, 14 * H:28 * H])
            for h in range(2):
                nc.scalar.dma_start(phs[h][:, 1, PH_SPLIT:NCH_P],
                                    ph_d[h][:, 1, PH_SPLIT:NCH_P])

            # --- PE warmup: dummy matmuls on zeroed scratch while input DMA
            # is in flight, so the HAM clock gate opens before real work ---
            wps = pint.tile([H, NTILE], f32, tag="ps")
            for w in range(N_WARM):
                nc.tensor.matmul(wps[:], warm[:, 0:H], warm[:],
                                 start=True, stop=True)

            pes = [[pedge.tile([H, SPC], f32, name=f"pe{side}_{ho}")
                    for ho in range(2)] for side in range(2)]

            def interior(ss):
                for tidx in range(NT):
                    t0 = tidx * NTILE
                    for ho in range(2):
                        ps = pint.tile([H, NTILE], f32, tag="ps")
                        for jp in range(7):
                            for h in range(2):
                                bidx = ho * 14 + jp * 2 + h
                                nc.tensor.matmul(
                                    ps[:],
                                    ktt[:, bidx * H:(bidx + 1) * H],
                                    phs[h][:, ss, t0 + jp: t0 + jp + NTILE],
                                    start=(jp == 0 and h == 0),
                                    stop=(jp == 6 and h == 1),
                                )
                        nc.vector.tensor_copy(outs[ss][ho][:, t0:t0 + NTILE], ps[:])

            def edges():
                for side in range(2):
                    for ho in range(2):
                        for jj in range(3):
                            for h in range(2):
                                eb = ((side * 2 + ho) * 3 + jj) * 2 + h
                                cp = (1 + jj) if side == 0 else (1202 + jj)
                                nc.tensor.matmul(
                                    pes[side][ho][:],
                                    et[:, eb * H:(eb + 1) * H],
                                    phs[h][:, :, cp],
                                    start=(jj == 0 and h == 0),
                                    stop=(jj == 2 and h == 1),
                                )

            def store(ss):
                eng = (nc.sync, nc.scalar)[ss]
                for ho in range(2):
                    nc.vector.tensor_add(
                        outs[ss][ho][:, 0:1], outs[ss][ho][:, 0:1],
                        pes[0][ho][:, ss:ss + 1])
                    nc.vector.tensor_add(
                        outs[ss][ho][:, NOUT - 1:NOUT], outs[ss][ho][:, NOUT - 1:NOUT],
                        pes[1][ho][:, ss:ss + 1])
                    eng.dma_start(out_d[ss, ho][:, 0:2 * NTILE],
                                  outs[ss][ho][:, 0:2 * NTILE])
                    eng.dma_start(out_d[ss, ho][:, 2 * NTILE:NOUT],
                                  outs[ss][ho][:, 2 * NTILE:NOUT])

            interior(0)
            edges()
            store(0)
            interior(1)
            store(1)

    nc.compile()
    _cache["nc"] = nc
    return nc


last_results = None  # BassKernelResults of the most recent run (for test harness)


def kernel(x, w_fwd_r=None, w_fwd_i=None, w_bwd_r=None, w_bwd_i=None):
    global last_results
    from concourse.bass_utils import run_bass_kernel_spmd

    x = np.asarray(x, dtype=np.float32)
    assert x.shape == (B, T), x.shape
    KTT, ET = _host_weights()
    nc = _build_nc()

    # Host marshalling: edge-pad, zero-chunk-pad, split chunks into halves,
    # transpose so the contraction index s' (0..99) is the partition dim.
    xp = np.pad(x, ((0, 0), (PAD, PAD)), mode="edge")       # [B, 240800]
    P = np.pad(xp, ((0, 0), (C, C))).astype(np.float16)     # [B, 241200]
    # ph[core][h, s', ss, c'] = P[2*core+ss, c'*200 + h*100 + s']
    P4 = P.reshape(B, NCH_P, 2, H)                          # (b, c', h, s')
    in_maps = []
    for core in range(N_CORES):
        blk = P4[core * SPC:(core + 1) * SPC]               # (ss, c', h, s')
        ph = np.ascontiguousarray(blk.transpose(2, 3, 0, 1))  # (h, s', ss, c')
        in_maps.append({"ph": ph, "ktt": KTT, "et": ET})

    res = run_bass_kernel_spmd(nc, in_maps, core_ids=list(range(N_CORES)))
    last_results = res

    y = np.empty((B, T), dtype=np.float32)
    for core in range(N_CORES):
        od = res.results[core]["out"]                       # [SPC, 2, H, NOUT]
        for ss in range(SPC):
            # y[t*200 + ho*100 + r'] = od[ss, ho, r', t]
            y[core * SPC + ss] = od[ss].transpose(2, 0, 1).reshape(T)
    return y


# revision 9
# speedup vs baseline: 1.5746x; 1.0718x over previous
"""Trainium2 Bass kernel for CustomSTFT (STFT -> mag/phase -> ISTFT roundtrip).

Key insight: magnitude*cos(atan2(i,r)) == r and magnitude*sin(atan2(i,r)) == i
(up to a 1e-14 epsilon inside the sqrt), so the whole pipeline is linear in x:

    y = OLA( frames @ M.T ),   M = w_bwd_r.T @ w_fwd_r - w_bwd_i.T @ w_fwd_i

With hop=200 and n_fft=800, collapsing the frame/OLA structure per 200-sample
chunk gives a 7-band polyphase filter: out_chunk[c] = sum_{d=-3..3} K_d @ in_chunk[c+d]
plus two boundary corrections (first/last output chunk miss one phantom frame).

Sharding: batch 16 across 8 cores (2 samples each). Matmuls in fp16 (weights
and data; f32 PSUM accumulation) — measured ~5e-4 scale-relative absmax.
"""

import numpy as np

N_CORES = 8
B, T = 16, 240000
SPC = B // N_CORES          # samples per core
N_FFT, HOP, FREQ, PAD = 800, 200, 401, 400
C = HOP                     # chunk size 200
H = C // 2                  # 100: SBUF partition dim (contraction split)
NCH_XP = (T + 2 * PAD) // C     # 1204 chunks in edge-padded signal
NCH_P = NCH_XP + 2              # 1206 incl. one zero chunk each side
NOUT = T // C                   # 1200 output chunks per sample
NTILE = 400                     # output chunks per matmul (psum free dim)
NT = NOUT // NTILE              # 3 interior tiles per sample
PH_SPLIT = NTILE + 6            # ph DMA piece boundary: cols [0,406) / [406,1206)
N_WARM = 12                     # dummy matmuls to warm the PE clock (HAM)

_cache = {}


def _host_weights():
    """Build the device weight tensors (fp16) from first principles."""
    if "ktt" in _cache:
        return _cache["ktt"], _cache["et"]
    n = np.arange(N_FFT)
    k = np.arange(FREQ)
    win = 0.5 * (1.0 - np.cos(2.0 * np.pi * np.arange(N_FFT) / N_FFT))
    angle = 2.0 * np.pi * np.outer(k, n) / N_FFT
    w_fwd_r = np.cos(angle) * win
    w_fwd_i = -np.sin(angle) * win
    inv_win = win / N_FFT
    w_bwd_r = np.cos(angle) * inv_win
    w_bwd_i = np.sin(angle) * inv_win
    M = w_bwd_r.T @ w_fwd_r - w_bwd_i.T @ w_fwd_i          # [800, 800] f64

    # Band kernels K_d[r, s] = sum_i M[i*C+r, (i+d)*C+s], d in [-3..3]
    # Kfull[r, q] with q = (d+3)*C + s -> out[t] = sum_{j'} K_{j'} @ Pch[t+j']
    Kfull = np.zeros((C, 7 * C))
    for d in range(-3, 4):
        for i in range(4):
            j = i + d
            if 0 <= j <= 3:
                Kfull[:, (d + 3) * C:(d + 4) * C] += M[i * C:(i + 1) * C, j * C:(j + 1) * C]

    # lhsT blocks for the interior matmuls, ho-major so the first psum groups
    # only gate on the first half of the weight DMA: b = ho*14 + jp*2 + h
    # KTT[s', b*H + r'] = Kfull[ho*H + r', jp*C + h*H + s']
    KTT = np.zeros((H, 28 * H), dtype=np.float16)
    for ho in range(2):
        for jp in range(7):
            for h in range(2):
                b = ho * 14 + jp * 2 + h
                blk = Kfull[ho * H:(ho + 1) * H, jp * C + h * H: jp * C + h * H + H]
                KTT[:, b * H:(b + 1) * H] = blk.T.astype(np.float16)

    # Edge corrections (pre-negated so they accumulate subtractively):
    # left  (out chunk t=0):    minus sum_{j=1..3} M[3C:4C, jC:(j+1)C] @ xp_chunk[j-1]
    # right (out chunk t=1199): minus sum_{j=0..2} M[0:C,  jC:(j+1)C] @ xp_chunk[1201+j]
    ET = np.zeros((H, 24 * H), dtype=np.float16)
    for side in range(2):
        for ho in range(2):
            for jj in range(3):
                for h in range(2):
                    if side == 0:
                        blk = M[3 * C + ho * H: 3 * C + (ho + 1) * H,
                                (jj + 1) * C + h * H: (jj + 1) * C + h * H + H]
                    else:
                        blk = M[ho * H:(ho + 1) * H,
                                jj * C + h * H: jj * C + h * H + H]
                    eb = ((side * 2 + ho) * 3 + jj) * 2 + h
                    ET[:, eb * H:(eb + 1) * H] = (-blk.T).astype(np.float16)

    _cache["ktt"], _cache["et"] = KTT, ET
    return KTT, ET


def _build_nc():
    if "nc" in _cache:
        return _cache["nc"]
    import concourse.mybir as mybir
    import concourse.tile as tile
    from concourse import bacc

    f32 = mybir.dt.float32
    f16 = mybir.dt.float16

    nc = bacc.Bacc("TRN2", target_bir_lowering=False, debug=False,
                   num_devices=N_CORES)
    ph_d = nc.dram_tensor("ph", [2, H, SPC, NCH_P], f16, kind="ExternalInput").ap()
    ktt_d = nc.dram_tensor("ktt", [H, 28 * H], f16, kind="ExternalInput").ap()
    et_d = nc.dram_tensor("et", [H, 24 * H], f16, kind="ExternalInput").ap()
    out_d = nc.dram_tensor("out", [SPC, 2, H, NOUT], f32, kind="ExternalOutput").ap()

    with tile.TileContext(nc) as tc:
        with (
            tc.tile_pool(name="weights", bufs=1) as wpool,
            tc.tile_pool(name="data", bufs=1) as dpool,
            tc.tile_pool(name="outp", bufs=1) as opool,
            tc.tile_pool(name="pint", bufs=4, space="PSUM") as pint,
            tc.tile_pool(name="pedge", bufs=1, space="PSUM") as pedge,
        ):
            ktt = wpool.tile([H, 28 * H], f16)
            et = wpool.tile([H, 24 * H], f16)
            warm = wpool.tile([H, NTILE], f16)
            phs = [dpool.tile([H, SPC, NCH_P], f16, name=f"ph{h}") for h in range(2)]
            outs = [[opool.tile([H, NOUT], f32, name=f"out{ss}_{ho}")
                     for ho in range(2)] for ss in range(SPC)]

            nc.gpsimd.memset(warm[:], 0.0)

            # --- input DMA, pieced for early compute start; triggers spread
            # across the two HWDGE lanes (sync, scalar); each trigger blocks
            # its lane ~0.9us. gpsimd SWDGE is slow — don't use it. ---
            for h in range(2):
                nc.sync.dma_start(phs[h][:, 0, 0:PH_SPLIT],
                                  ph_d[h][:, 0, 0:PH_SPLIT])
            nc.scalar.dma_start(ktt[:, 0:14 * H], ktt_d[:, 0:14 * H])
            nc.scalar.dma_start(ktt[:, 14 * H:28 * H], ktt_d[:, 14 * H:28 * H])
            for h in range(2):
                nc.sync.dma_start(phs[h][:, 0, PH_SPLIT:NCH_P],
                                  ph_d[h][:, 0, PH_SPLIT:NCH_P])
            nc.sync.dma_start(et[:], et_d[:])
            for h in range(2):
                nc.scalar.dma_start(phs[h][:, 1, 0:PH_SPLIT],
                                    ph_d[h][:, 1, 0:PH_SPLIT])
            for h in range(2):
                nc.scalar.dma_start(phs[h][:, 1, PH_SPLIT:NCH_P],
                                    ph_d[h][:, 1, PH_SPLIT:NCH_P])

            # --- PE warmup: dummy matmuls on zeroed scratch while input DMA
            # is in flight, so the HAM clock gate opens before real work ---
            wps = pint.tile([H, NTILE], f32, tag="ps")
            for w in range(N_WARM):
                nc.tensor.matmul(wps[:], warm[:, 0:H], warm[:],
                                 start=True, stop=True)

            pes = [[pedge.tile([H, SPC], f32, name=f"pe{side}_{ho}")
                    for ho in range(2)] for side in range(2)]

            def interior(ss, ho):
                for tidx in range(NT):
                    t0 = tidx * NTILE
                    ps = pint.tile([H, NTILE], f32, tag="ps")
                    for jp in range(7):
                        for h in range(2):
                            bidx = ho * 14 + jp * 2 + h
                            nc.tensor.matmul(
                                ps[:],
                                ktt[:, bidx * H:(bidx + 1) * H],
                                phs[h][:, ss, t0 + jp: t0 + jp + NTILE],
                                start=(jp == 0 and h == 0),
                                stop=(jp == 6 and h == 1),
                            )
                    nc.vector.tensor_copy(outs[ss][ho][:, t0:t0 + NTILE], ps[:])

            def edges():
                for side in range(2):
                    for ho in range(2):
                        for jj in range(3):
                            for h in range(2):
                                eb = ((side * 2 + ho) * 3 + jj) * 2 + h
                                cp = (1 + jj) if side == 0 else (1202 + jj)
                                nc.tensor.matmul(
                                    pes[side][ho][:],
                                    et[:, eb * H:(eb + 1) * H],
                                    phs[h][:, :, cp],
                                    start=(jj == 0 and h == 0),
                                    stop=(jj == 2 and h == 1),
                                )

            def store(ss, ho):
                eng = (nc.sync, nc.scalar)[ss]
                nc.vector.tensor_add(
                    outs[ss][ho][:, 0:1], outs[ss][ho][:, 0:1],
                    pes[0][ho][:, ss:ss + 1])
                nc.vector.tensor_add(
                    outs[ss][ho][:, NOUT - 1:NOUT], outs[ss][ho][:, NOUT - 1:NOUT],
                    pes[1][ho][:, ss:ss + 1])
                eng.dma_start(out_d[ss, ho][:, 0:2 * NTILE],
                              outs[ss][ho][:, 0:2 * NTILE])
                eng.dma_start(out_d[ss, ho][:, 2 * NTILE:NOUT],
                              outs[ss][ho][:, 2 * NTILE:NOUT])

            interior(0, 0)
            interior(0, 1)
            edges()
            store(0, 0)
            store(0, 1)
            interior(1, 0)
            store(1, 0)
            interior(1, 1)
            store(1, 1)

    nc.compile()
    _cache["nc"] = nc
    return nc


last_results = None  # BassKernelResults of the most recent run (for test harness)


def kernel(x, w_fwd_r=None, w_fwd_i=None, w_bwd_r=None, w_bwd_i=None):
    global last_results
    from concourse.bass_utils import run_bass_kernel_spmd

    x = np.asarray(x, dtype=np.float32)
    assert x.shape == (B, T), x.shape
    KTT, ET = _host_weights()
    nc = _build_nc()

    # Host marshalling: edge-pad, zero-chunk-pad, split chunks into halves,
    # transpose so the contraction index s' (0..99) is the partition dim.
    xp = np.pad(x, ((0, 0), (PAD, PAD)), mode="edge")       # [B, 240800]
    P = np.pad(xp, ((0, 0), (C, C))).astype(np.float16)     # [B, 241200]
    # ph[core][h, s', ss, c'] = P[2*core+ss, c'*200 + h*100 + s']
    P4 = P.reshape(B, NCH_P, 2, H)                          # (b, c', h, s')
    in_maps = []
    for core in range(N_CORES):
        blk = P4[core * SPC:(core + 1) * SPC]               # (ss, c', h, s')
        ph = np.ascontiguousarray(blk.transpose(2, 3, 0, 1))  # (h, s', ss, c')
        in_maps.append({"ph": ph, "ktt": KTT, "et": ET})

    res = run_bass_kernel_spmd(nc, in_maps, core_ids=list(range(N_CORES)))
    last_results = res

    y = np.empty((B, T), dtype=np.float32)
    for core in range(N_CORES):
        od = res.results[core]["out"]                       # [SPC, 2, H, NOUT]
        for ss in range(SPC):
            # y[t*200 + ho*100 + r'] = od[ss, ho, r', t]
            y[core * SPC + ss] = od[ss].transpose(2, 0, 1).reshape(T)
    return y
